# revision 20
# baseline (speedup 1.0000x reference)
"""BalanceMSELoss on 8 Trainium2 NeuronCores — v7 (all-fp8, DVE-only).

reference:
    inside = box mask from boxes (per batch), d2 = (input-target)^2
    loss = 0.5 * sum(d2*m)/sum(m) + 0.5 * sum(d2*(1-m))/sum(1-m)
    (the reference mask is (B,1,H,W): its count excludes the C factor
     while its masked sum spans all C channels)

Sharding: batch dim B=32 -> 8 cores x 4 images (data parallel).

All input data is staged fp8e4m3 (the 2e-2 rel-err budget dwarfs the
~1.5e-3 quantization error): 6.3 MiB/core of DMA.  All elementwise work
runs as one fused DIFF_SQ custom-DVE pass (f8 in, f8 out, 1.03
cyc/elem = 552 ns per 64Ki-element col-group), 26.5 us total on DVE —
measured to run back-to-back at clean rate while DMA + PE stream
concurrently.  Design history (v4-v6 traces):
  - GpSimd running concurrently degrades every other engine 1.5-2x
    (SBUF contention) — pool is not used at all;
  - splitting work to ACT via f16 staging (sub 2x-mode + ACT square)
    runs clean but the extra SBUF bytes (2x DMA + f16 diff write/read)
    hit an ~800 GB/s aggregate SBUF-traffic wall — net wash (43.9 us
    vs 43.4 us all-f8);
  - sliced sub-tile operands / PE matmuls interleaved into a tile the
    producer is still writing run ~2.2x slower: ops are whole-tile and
    each chunk's matmuls are emitted after its d2 tile completes.

Per-core layout: the 4 images' [C*H, W] = [6144, 512] rows are assigned
48-per-partition (rows 48p..48p+47).  Chunk ci covers col-groups
[off, off+n) in stream order, one DMA each on the qSP ring: per
partition [2(tensor), n(row), 512] fp8 contiguous DRAM lines (inp rows,
tgt rows).  A 2-cg chunk opens (first matmul by ~2 us); 4- and 2-cg
chunks close (short drain chain: fused -> matmul -> PSUM copy -> out).

Compute per pair (2 col-groups): one DoubleRow fp8 matmul
lhsT=ym[:, m, :, :8] ([128, 2, 8]: per image b, cols (2b, 2b+1) hold
(ymask*indicator, indicator), nonzero only on partitions 32b..32b+31)
x d2[:, pair, :] accumulated into one PSUM tile [8, 512] f32 across all
24 pairs. Rows 2b = y-masked column sums, 2b+1 = unmasked, per image.
Host applies the x-direction mask: ins_b = dot(ps[2b], xmask_b);
tot_b = sum(ps[2b+1]). Host reduces in float64 and forms the loss.
"""

import numpy as np

N_CORES = 8
B, C, H, W = 32, 3, 512, 512
BL = B // N_CORES        # images per core
P = 128
RPP = BL * C * H // P    # rows per partition = 48
# (dtype, col-groups) chunks in stream order.  All-f8, fused DIFF_SQ on
# DVE only: traces across five variants show the system is pinned
# between the DVE cycle floor (48 cg x 552 ns = 26.5 us single pass)
# and an ~800 GB/s aggregate SBUF-traffic wall.  The f8-fused path
# moves the fewest SBUF bytes per cg (DMA 128K + DVE 192K + PE 64K =
# 384K vs 896K for f16 sub+ACT square), so all-f8 sits under the wall
# and the DVE floor is the binder.  GpSimd is not used at all: a
# concurrent pool op degrades every other engine 1.5-2x.  A tiny chunk
# leads (first matmul by ~2us) and small chunks close (short drain).
CHUNKS = [("f8", 2), ("f8", 8), ("f16", 6), ("f8", 8), ("f8", 8),
          ("f8", 8), ("f8", 6), ("f8", 2)]
ETA = 1.0

_CACHE = {}


def _ensure_diff_sq():
    """Register a fused (a-b)^2 custom DVE op via the documented extension
    flow (dve_ops.py: 'Adding a new op: define a DveOp constant and append
    it to OPS'). Idempotent; the uops sha is pinned from lower() output."""
    import numpy as np
    import concourse.dve_ops as dve_ops
    from concourse.dve_spec import Spec, Src0, Src1, lower, sq, _has_src1
    from concourse.dve_uop import DveOpSpec
    from concourse.dve_table_gen import dve_ver_for

    name = "DIFF_SQ_ANT"
    for op in dve_ops.OPS:
        if op.name == name:
            return op
    spec = Spec(
        body=sq(Src0 - Src1),
        reference=lambda in0, in1, s0, s1, imm2: (
            (in0.astype(np.float32) - in1) ** 2),
    )
    op = dve_ops.DveOp(name, spec, subdim=False, uops_sha={})
    row = dve_ops._CUSTOM_DVE_ROW_BASE + len(dve_ops.OPS)
    assert row < 0x20
    dve_ops.OPS.append(op)
    dve_ops._SUB_OPCODE_FOR_NAME[name] = row
    dve_ops.CUSTOM_DVE_SPECS[name] = spec
    ver = dve_ver_for("TRN2")
    uops = lower(spec, ver=ver)
    op.uops_sha[ver] = DveOpSpec(
        name=name, opcode=row, uops=uops, rd1_en=_has_src1(spec)).sha(ver)
    return op


def _build_program():
    import concourse.bacc as bacc
    import concourse.mybir as mybir
    import concourse.tile as tile

    diff_sq = _ensure_diff_sq()

    nc = bacc.Bacc("TRN2", debug=False, target_bir_lowering=False,
                   num_devices=N_CORES)
    f32 = mybir.dt.float32
    f16 = mybir.dt.float16
    f8 = mybir.dt.float8e4
    ncg = sum(n for _, n in CHUNKS)
    assert ncg == RPP
    n8 = sum(n for d, n in CHUNKS if d == "f8")
    n16 = sum(n for d, n in CHUNKS if d == "f16")
    cat8 = nc.dram_tensor("cat8", [P, n8 * 2 * W], f8, kind="ExternalInput")
    cat16 = (nc.dram_tensor("cat16", [P, n16 * 2 * W], f16,
                            kind="ExternalInput") if n16 else None)
    # [P, RPP//2, 2, 16] fp8 (cols padded 8->16 so the DoubleRow pair-dim
    # byte-step is 16, per the ISA LDWEIGHTS constraint): pair m ->
    # lhsT [128, 2, 8] via [:, m, :, :8]
    ym = nc.dram_tensor("ym", [P, RPP // 2, 2, 16], f8, kind="ExternalInput")
    # [8, 512] f32: rows 2b = y-masked col sums of image b, 2b+1 = unmasked
    ycols = nc.dram_tensor("ycols", [8, W], f32, kind="ExternalOutput")

    cat8_ap = cat8.ap()
    cat16_ap = cat16.ap() if cat16 is not None else None
    ycols_ap = ycols.ap()

    with tile.TileContext(nc) as tc:
        with tc.tile_pool(name="singles", bufs=1) as singles, \
             tc.tile_pool(name="diffs", bufs=1) as diffs, \
             tc.tile_pool(name="psum", bufs=1, space="PSUM") as psum_pool:
            ymt = singles.tile([P, RPP // 2, 2, 16], f8)
            # ym rides qAct so qSP carries only the 8 input chunks + out
            nc.scalar.dma_start(out=ymt, in_=ym.ap())

            # All input DMAs issue upfront: f8 chunks on the qSP ring
            # (nc.sync), f16 chunks on qAct (nc.scalar), each ring in
            # consumption order (<= 8 outstanding per ring).
            tiles = []
            off8 = off16 = 0
            for ci, (dt_, n) in enumerate(CHUNKS):
                if dt_ == "f8":
                    t = singles.tile([P, 2, n, W], f8, tag=f"io{ci}")
                    nc.sync.dma_start(
                        out=t,
                        in_=cat8_ap[:, off8 * 2 * W:(off8 + n) * 2 * W]
                        .rearrange("p (t n w) -> p t n w", t=2, n=n, w=W))
                    off8 += n
                else:
                    t = singles.tile([P, 2, n, W], f16, tag=f"io{ci}")
                    nc.scalar.dma_start(
                        out=t,
                        in_=cat16_ap[:, off16 * 2 * W:(off16 + n) * 2 * W]
                        .rearrange("p (t n w) -> p t n w", t=2, n=n, w=W))
                    off16 += n
                d2 = singles.tile([P, n, W], f8, tag=f"d2{ci}")
                tiles.append((t, d2))

            ps = psum_pool.tile([8, W], f32)
            off = 0
            for ci, (dt_, n) in enumerate(CHUNKS):
                t, d2 = tiles[ci]
                if dt_ == "f16":
                    # whole-chunk DVE sub (2x_1P), whole-chunk ACT square
                    diff = diffs.tile([P, n, W], f16, tag=f"diff{ci}")
                    nc.vector.tensor_sub(diff, t[:, 0], t[:, 1])
                    nc.scalar.square(d2, diff)
                else:
                    nc.vector._custom_dve(
                        diff_sq, out=d2, in0=t[:, 0], in1=t[:, 1])
                for q2 in range(0, n, 2):
                    m = (off + q2) // 2
                    nc.tensor.matmul(
                        ps[:], ymt[:, m, :, :8], d2[:, q2:q2 + 2, :],
                        start=(m == 0), stop=(m == RPP // 2 - 1),
                        perf_mode=mybir.MatmulPerfMode.DoubleRow)
                off += n

            # PSUM can't be a DMA source; bounce through SBUF (DVE is free
            # right after the tail chunk's fused op).
            sb = singles.tile([8, W], f32, tag="psout")
            nc.vector.tensor_copy(sb, ps[:])
            # out-DMA on qSP: idle since the input issues, so its
            # descriptor-gen pre-processes while waiting on the copy sem.
            nc.sync.dma_start(out=ycols_ap, in_=sb[:])

    nc.compile()
    return nc


def _get_exec():
    """Build program once and wrap it in a cached jitted shard_map callable."""
    if "exec" in _CACHE:
        return _CACHE["exec"]
    import jax
    from jax.experimental.shard_map import shard_map
    from jax.sharding import Mesh, PartitionSpec

    import concourse.mybir as mybir
    from concourse import bass2jax

    nc = _build_program()
    bass2jax.install_neuronx_cc_hook()

    partition_name = nc.partition_id_tensor.name if nc.partition_id_tensor else None
    in_names, out_names, out_avals = [], [], []
    for alloc in nc.m.functions[0].allocations:
        if not isinstance(alloc, mybir.MemoryLocationSet):
            continue
        name = alloc.memorylocations[0].name
        if alloc.kind == "ExternalInput":
            if name != partition_name:
                in_names.append(name)
        elif alloc.kind == "ExternalOutput":
            out_avals.append(jax.core.ShapedArray(
                tuple(alloc.tensor_shape), mybir.dt.np(alloc.dtype)))
            out_names.append(name)
    n_params = len(in_names)
    n_outs = len(out_names)
    all_in_names = in_names + out_names + (
        [partition_name] if partition_name else [])
    donate = tuple(range(n_params, n_params + n_outs))

    def _body(*args):
        operands = list(args)
        if partition_name is not None:
            operands.append(bass2jax.partition_id_tensor())
        return tuple(bass2jax._bass_exec_p.bind(
            *operands,
            out_avals=tuple(out_avals),
            in_names=tuple(all_in_names),
            out_names=tuple(out_names),
            lowering_input_output_aliases=(),
            sim_require_finite=True,
            sim_require_nnan=True,
            nc=nc,
        ))

    devices = jax.devices()[:N_CORES]
    assert len(devices) == N_CORES
    mesh = Mesh(np.asarray(devices), ("core",))
    in_specs = (PartitionSpec("core"),) * (n_params + n_outs)
    out_specs = (PartitionSpec("core"),) * n_outs
    sharded = jax.jit(
        shard_map(_body, mesh=mesh, in_specs=in_specs, out_specs=out_specs,
                  check_rep=False),
        donate_argnums=donate, keep_unused=True)

    ex = dict(nc=nc, sharded=sharded, in_names=in_names, out_names=out_names,
              out_avals=out_avals, mesh=mesh, n_params=n_params)
    _CACHE["exec"] = ex
    return ex


def _prepare(input, target, boxes):
    """Host-side prep: fp8 staging layout, masks, counts."""
    input = np.asarray(input, dtype=np.float32)
    target = np.asarray(target, dtype=np.float32)
    boxes = np.asarray(boxes, dtype=np.float32)

    # Box coordinates exactly as the reference computes them (f32 multiply,
    # floor, int cast).
    x1 = np.floor(boxes[:, 0] * np.float32(W)).astype(np.int64)
    y1 = np.floor(boxes[:, 1] * np.float32(H)).astype(np.int64)
    bw = np.floor(boxes[:, 2] * np.float32(W)).astype(np.int64)
    bh = np.floor(boxes[:, 3] * np.float32(H)).astype(np.int64)

    xs = np.arange(W)
    ys = np.arange(H)
    xmask_full = ((xs[None, :] >= x1[:, None]) &
                  (xs[None, :] <= (x1 + bw)[:, None])).astype(np.float64)
    ymask_full = ((ys[None, :] >= y1[:, None]) &
                  (ys[None, :] <= (y1 + bh)[:, None])).astype(np.float64)

    # cat layout [cores, P, chunk ci: [2, n, W]]: partition p holds rows
    # 48p..48p+47 of the per-core [BL*C*H, W] stack; chunk ci covers
    # within-partition col-groups [off, off+n), inp rows then tgt rows.
    import ml_dtypes
    f8np = ml_dtypes.float8_e4m3  # TRN fp8e4 variant (max normal 240)
    inp_r = input.reshape(N_CORES, P, RPP, W)
    tgt_r = target.reshape(N_CORES, P, RPP, W)
    n8 = sum(n for d, n in CHUNKS if d == "f8")
    n16 = sum(n for d, n in CHUNKS if d == "f16")
    cat8 = np.empty((N_CORES, P, n8 * 2 * W), dtype=f8np)
    cat16 = np.empty((N_CORES, P, n16 * 2 * W), dtype=np.float16)
    # (cat16 is skipped below and in concat when the schedule has no f16)
    pos8 = pos16 = 0
    off = 0
    for dt_, n in CHUNKS:
        if dt_ == "f8":
            cat8[:, :, pos8:pos8 + n * W] = inp_r[:, :, off:off + n
                ].reshape(N_CORES, P, n * W).astype(f8np)
            cat8[:, :, pos8 + n * W:pos8 + 2 * n * W] = tgt_r[:, :, off:off + n
                ].reshape(N_CORES, P, n * W).astype(f8np)
            pos8 += 2 * n * W
        else:
            cat16[:, :, pos16:pos16 + n * W] = inp_r[:, :, off:off + n
                ].reshape(N_CORES, P, n * W).astype(np.float16)
            cat16[:, :, pos16 + n * W:pos16 + 2 * n * W] = tgt_r[:, :, off:off + n
                ].reshape(N_CORES, P, n * W).astype(np.float16)
            pos16 += 2 * n * W
        off += n

    # ym [cores, P, RPP, 8] fp32: for col-group jj at partition p, global
    # row g = 48p + jj -> image b = g // (C*H) (== p // 32), h = g % H.
    # col 2b = ymask(h), col 2b+1 = 1.0 (only for the owning image's cols).
    pp, jj = np.meshgrid(np.arange(P), np.arange(RPP), indexing="ij")
    g = RPP * pp + jj                      # [P, RPP]
    b_idx = g // (C * H)                   # [P, RPP] in 0..BL-1
    h_idx = g % H
    ym = np.zeros((N_CORES, P, RPP, 8), dtype=np.float32)
    for k in range(N_CORES):
        ymv = ymask_full[4 * k + b_idx, h_idx]       # [P, RPP]
        np.put_along_axis(ym[k], (2 * b_idx)[..., None],
                          ymv[..., None].astype(np.float32), axis=2)
        np.put_along_axis(ym[k], (2 * b_idx + 1)[..., None],
                          np.float32(1.0), axis=2)
    # DoubleRow pair layout [cores, P, RPP//2, 2, 16], fp8 (0/1 exact),
    # cols padded to 16 for the ISA pair-step constraint
    ymp = np.zeros((N_CORES, P, RPP // 2, 2, 16), dtype=f8np)
    ymp[..., :8] = ym.reshape(N_CORES, P, RPP // 2, 2, 8).astype(f8np)
    ym = ymp

    concat = {
        "cat8": np.ascontiguousarray(cat8.reshape(N_CORES * P, n8 * 2 * W)),
        "ym": np.ascontiguousarray(ym.reshape(N_CORES * P, RPP // 2, 2, 16)),
    }
    if n16:
        concat["cat16"] = np.ascontiguousarray(
            cat16.reshape(N_CORES * P, n16 * 2 * W))

    # NB: reference mask is (B,1,H,W) — counts exclude the C factor.
    ins_cnt = float((xmask_full.sum(axis=1) * ymask_full.sum(axis=1)).sum())
    tot_cnt = float(B * H * W)
    return concat, ins_cnt, tot_cnt, xmask_full


def _run(ex, concat):
    import jax
    concat_in = [concat[name] for name in ex["in_names"]]
    zeros = [np.zeros((N_CORES * av.shape[0], *av.shape[1:]), av.dtype)
             for av in ex["out_avals"]]
    out_arrs = ex["sharded"](*concat_in, *zeros)
    out_arrs = jax.block_until_ready(out_arrs)
    return {name: np.asarray(out_arrs[i])
            for i, name in enumerate(ex["out_names"])}


def kernel(input, target, boxes):
    ex = _get_exec()
    concat, ins_cnt, tot_cnt, xmask_full = _prepare(input, target, boxes)
    outs = _run(ex, concat)

    # ycols global [8*cores, 512]: core k rows 8k+2b / 8k+2b+1 = image 4k+b
    yc = outs["ycols"].astype(np.float64).reshape(N_CORES, BL, 2, W)
    ymasked = yc[:, :, 0, :].reshape(B, W)
    tot = yc[:, :, 1, :].reshape(B, W)
    ins_sum = float((ymasked * xmask_full).sum())
    tot_sum = float(tot.sum())

    inside_loss = ins_sum / ins_cnt
    outside_loss = (tot_sum - ins_sum) / (tot_cnt - ins_cnt)
    loss = (0.5 * inside_loss + 0.5 * outside_loss) * ETA
    return np.asarray(loss, dtype=np.float32)


# revision 21
# speedup vs baseline: 1.1839x; 1.1839x over previous
"""BalanceMSELoss on 8 Trainium2 NeuronCores — v7 (all-fp8, DVE-only).

reference:
    inside = box mask from boxes (per batch), d2 = (input-target)^2
    loss = 0.5 * sum(d2*m)/sum(m) + 0.5 * sum(d2*(1-m))/sum(1-m)
    (the reference mask is (B,1,H,W): its count excludes the C factor
     while its masked sum spans all C channels)

Sharding: batch dim B=32 -> 8 cores x 4 images (data parallel).

All input data is staged fp8e4m3 (the 2e-2 rel-err budget dwarfs the
~1.5e-3 quantization error): 6.3 MiB/core of DMA.  All elementwise work
runs as one fused DIFF_SQ custom-DVE pass (f8 in, f8 out, 1.03
cyc/elem = 552 ns per 64Ki-element col-group), 26.5 us total on DVE —
measured to run back-to-back at clean rate while DMA + PE stream
concurrently.  Design history (v4-v6 traces):
  - GpSimd running concurrently degrades every other engine 1.5-2x
    (SBUF contention) — pool is not used at all;
  - splitting work to ACT via f16 staging (sub 2x-mode + ACT square)
    runs clean but the extra SBUF bytes (2x DMA + f16 diff write/read)
    hit an ~800 GB/s aggregate SBUF-traffic wall — net wash (43.9 us
    vs 43.4 us all-f8);
  - sliced sub-tile operands / PE matmuls interleaved into a tile the
    producer is still writing run ~2.2x slower: ops are whole-tile and
    each chunk's matmuls are emitted after its d2 tile completes.

Per-core layout: the 4 images' [C*H, W] = [6144, 512] rows are assigned
48-per-partition (rows 48p..48p+47).  Chunk ci covers col-groups
[off, off+n) in stream order, one DMA each on the qSP ring: per
partition [2(tensor), n(row), 512] fp8 contiguous DRAM lines (inp rows,
tgt rows).  A 2-cg chunk opens (first matmul by ~2 us); 4- and 2-cg
chunks close (short drain chain: fused -> matmul -> PSUM copy -> out).

Compute per pair (2 col-groups): one DoubleRow fp8 matmul
lhsT=ym[:, m, :, :8] ([128, 2, 8]: per image b, cols (2b, 2b+1) hold
(ymask*indicator, indicator), nonzero only on partitions 32b..32b+31)
x d2[:, pair, :] accumulated into one PSUM tile [8, 512] f32 across all
24 pairs. Rows 2b = y-masked column sums, 2b+1 = unmasked, per image.
Host applies the x-direction mask: ins_b = dot(ps[2b], xmask_b);
tot_b = sum(ps[2b+1]). Host reduces in float64 and forms the loss.
"""

import numpy as np

N_CORES = 8
B, C, H, W = 32, 3, 512, 512
BL = B // N_CORES        # images per core
P = 128
RPP = BL * C * H // P    # rows per partition = 48
# (dtype, col-groups) chunks in stream order.  All-f8, fused DIFF_SQ on
# DVE only: traces across five variants show the system is pinned
# between the DVE cycle floor (48 cg x 552 ns = 26.5 us single pass)
# and an ~800 GB/s aggregate SBUF-traffic wall.  The f8-fused path
# moves the fewest SBUF bytes per cg (DMA 128K + DVE 192K + PE 64K =
# 384K vs 896K for f16 sub+ACT square), so all-f8 sits under the wall
# and the DVE floor is the binder.  GpSimd is not used at all: a
# concurrent pool op degrades every other engine 1.5-2x.  A tiny chunk
# leads (first matmul by ~2us) and small chunks close (short drain).
CHUNKS = [("f8", 2), ("f8", 8), ("f8", 8), ("f8", 8), ("f8", 8),
          ("f8", 8), ("f8", 4), ("f8", 2)]
ETA = 1.0

_CACHE = {}


def _ensure_diff_sq():
    """Register a fused (a-b)^2 custom DVE op via the documented extension
    flow (dve_ops.py: 'Adding a new op: define a DveOp constant and append
    it to OPS'). Idempotent; the uops sha is pinned from lower() output."""
    import numpy as np
    import concourse.dve_ops as dve_ops
    from concourse.dve_spec import Spec, Src0, Src1, lower, sq, _has_src1
    from concourse.dve_uop import DveOpSpec
    from concourse.dve_table_gen import dve_ver_for

    name = "DIFF_SQ_ANT"
    for op in dve_ops.OPS:
        if op.name == name:
            return op
    spec = Spec(
        body=sq(Src0 - Src1),
        reference=lambda in0, in1, s0, s1, imm2: (
            (in0.astype(np.float32) - in1) ** 2),
    )
    op = dve_ops.DveOp(name, spec, subdim=False, uops_sha={})
    row = dve_ops._CUSTOM_DVE_ROW_BASE + len(dve_ops.OPS)
    assert row < 0x20
    dve_ops.OPS.append(op)
    dve_ops._SUB_OPCODE_FOR_NAME[name] = row
    dve_ops.CUSTOM_DVE_SPECS[name] = spec
    ver = dve_ver_for("TRN2")
    uops = lower(spec, ver=ver)
    op.uops_sha[ver] = DveOpSpec(
        name=name, opcode=row, uops=uops, rd1_en=_has_src1(spec)).sha(ver)
    return op


def _build_program():
    import concourse.bacc as bacc
    import concourse.mybir as mybir
    import concourse.tile as tile

    diff_sq = _ensure_diff_sq()

    nc = bacc.Bacc("TRN2", debug=False, target_bir_lowering=False,
                   num_devices=N_CORES)
    f32 = mybir.dt.float32
    f16 = mybir.dt.float16
    f8 = mybir.dt.float8e4
    ncg = sum(n for _, n in CHUNKS)
    assert ncg == RPP
    n8 = sum(n for d, n in CHUNKS if d == "f8")
    n16 = sum(n for d, n in CHUNKS if d == "f16")
    cat8 = nc.dram_tensor("cat8", [P, n8 * 2 * W], f8, kind="ExternalInput")
    cat16 = (nc.dram_tensor("cat16", [P, n16 * 2 * W], f16,
                            kind="ExternalInput") if n16 else None)
    # [P, RPP//2, 2, 16] fp8 (cols padded 8->16 so the DoubleRow pair-dim
    # byte-step is 16, per the ISA LDWEIGHTS constraint): pair m ->
    # lhsT [128, 2, 8] via [:, m, :, :8]
    ym = nc.dram_tensor("ym", [P, RPP // 2, 2, 16], f8, kind="ExternalInput")
    # [8, 512] f32: rows 2b = y-masked col sums of image b, 2b+1 = unmasked
    ycols = nc.dram_tensor("ycols", [8, W], f32, kind="ExternalOutput")

    cat8_ap = cat8.ap()
    cat16_ap = cat16.ap() if cat16 is not None else None
    ycols_ap = ycols.ap()

    with tile.TileContext(nc) as tc:
        with tc.tile_pool(name="singles", bufs=1) as singles, \
             tc.tile_pool(name="diffs", bufs=1) as diffs, \
             tc.tile_pool(name="psum", bufs=1, space="PSUM") as psum_pool:
            ymt = singles.tile([P, RPP // 2, 2, 16], f8)
            # ym rides qAct so qSP carries only the 8 input chunks + out
            nc.scalar.dma_start(out=ymt, in_=ym.ap())

            # All input DMAs issue upfront: f8 chunks on the qSP ring
            # (nc.sync), f16 chunks on qAct (nc.scalar), each ring in
            # consumption order (<= 8 outstanding per ring).
            tiles = []
            off8 = off16 = 0
            for ci, (dt_, n) in enumerate(CHUNKS):
                if dt_ == "f8":
                    t = singles.tile([P, 2, n, W], f8, tag=f"io{ci}")
                    nc.sync.dma_start(
                        out=t,
                        in_=cat8_ap[:, off8 * 2 * W:(off8 + n) * 2 * W]
                        .rearrange("p (t n w) -> p t n w", t=2, n=n, w=W))
                    off8 += n
                else:
                    t = singles.tile([P, 2, n, W], f16, tag=f"io{ci}")
                    nc.scalar.dma_start(
                        out=t,
                        in_=cat16_ap[:, off16 * 2 * W:(off16 + n) * 2 * W]
                        .rearrange("p (t n w) -> p t n w", t=2, n=n, w=W))
                    off16 += n
                d2 = singles.tile([P, n, W], f8, tag=f"d2{ci}")
                tiles.append((t, d2))

            ps = psum_pool.tile([8, W], f32)
            off = 0
            for ci, (dt_, n) in enumerate(CHUNKS):
                t, d2 = tiles[ci]
                if dt_ == "f16":
                    # whole-chunk DVE sub (2x_1P), whole-chunk ACT square
                    diff = diffs.tile([P, n, W], f16, tag=f"diff{ci}")
                    nc.vector.tensor_sub(diff, t[:, 0], t[:, 1])
                    nc.scalar.square(d2, diff)
                else:
                    nc.vector._custom_dve(
                        diff_sq, out=d2, in0=t[:, 0], in1=t[:, 1])
                for q2 in range(0, n, 2):
                    m = (off + q2) // 2
                    nc.tensor.matmul(
                        ps[:], ymt[:, m, :, :8], d2[:, q2:q2 + 2, :],
                        start=(m == 0), stop=(m == RPP // 2 - 1),
                        perf_mode=mybir.MatmulPerfMode.DoubleRow)
                off += n

            # PSUM can't be a DMA source; bounce through SBUF (DVE is free
            # right after the tail chunk's fused op).
            sb = singles.tile([8, W], f32, tag="psout")
            nc.vector.tensor_copy(sb, ps[:])
            # out-DMA on qSP: idle since the input issues, so its
            # descriptor-gen pre-processes while waiting on the copy sem.
            nc.sync.dma_start(out=ycols_ap, in_=sb[:])

    nc.compile()
    return nc


def _get_exec():
    """Build program once and wrap it in a cached jitted shard_map callable."""
    if "exec" in _CACHE:
        return _CACHE["exec"]
    import jax
    from jax.experimental.shard_map import shard_map
    from jax.sharding import Mesh, PartitionSpec

    import concourse.mybir as mybir
    from concourse import bass2jax

    nc = _build_program()
    bass2jax.install_neuronx_cc_hook()

    partition_name = nc.partition_id_tensor.name if nc.partition_id_tensor else None
    in_names, out_names, out_avals = [], [], []
    for alloc in nc.m.functions[0].allocations:
        if not isinstance(alloc, mybir.MemoryLocationSet):
            continue
        name = alloc.memorylocations[0].name
        if alloc.kind == "ExternalInput":
            if name != partition_name:
                in_names.append(name)
        elif alloc.kind == "ExternalOutput":
            out_avals.append(jax.core.ShapedArray(
                tuple(alloc.tensor_shape), mybir.dt.np(alloc.dtype)))
            out_names.append(name)
    n_params = len(in_names)
    n_outs = len(out_names)
    all_in_names = in_names + out_names + (
        [partition_name] if partition_name else [])
    donate = tuple(range(n_params, n_params + n_outs))

    def _body(*args):
        operands = list(args)
        if partition_name is not None:
            operands.append(bass2jax.partition_id_tensor())
        return tuple(bass2jax._bass_exec_p.bind(
            *operands,
            out_avals=tuple(out_avals),
            in_names=tuple(all_in_names),
            out_names=tuple(out_names),
            lowering_input_output_aliases=(),
            sim_require_finite=True,
            sim_require_nnan=True,
            nc=nc,
        ))

    devices = jax.devices()[:N_CORES]
    assert len(devices) == N_CORES
    mesh = Mesh(np.asarray(devices), ("core",))
    in_specs = (PartitionSpec("core"),) * (n_params + n_outs)
    out_specs = (PartitionSpec("core"),) * n_outs
    sharded = jax.jit(
        shard_map(_body, mesh=mesh, in_specs=in_specs, out_specs=out_specs,
                  check_rep=False),
        donate_argnums=donate, keep_unused=True)

    ex = dict(nc=nc, sharded=sharded, in_names=in_names, out_names=out_names,
              out_avals=out_avals, mesh=mesh, n_params=n_params)
    _CACHE["exec"] = ex
    return ex


def _prepare(input, target, boxes):
    """Host-side prep: fp8 staging layout, masks, counts."""
    input = np.asarray(input, dtype=np.float32)
    target = np.asarray(target, dtype=np.float32)
    boxes = np.asarray(boxes, dtype=np.float32)

    # Box coordinates exactly as the reference computes them (f32 multiply,
    # floor, int cast).
    x1 = np.floor(boxes[:, 0] * np.float32(W)).astype(np.int64)
    y1 = np.floor(boxes[:, 1] * np.float32(H)).astype(np.int64)
    bw = np.floor(boxes[:, 2] * np.float32(W)).astype(np.int64)
    bh = np.floor(boxes[:, 3] * np.float32(H)).astype(np.int64)

    xs = np.arange(W)
    ys = np.arange(H)
    xmask_full = ((xs[None, :] >= x1[:, None]) &
                  (xs[None, :] <= (x1 + bw)[:, None])).astype(np.float64)
    ymask_full = ((ys[None, :] >= y1[:, None]) &
                  (ys[None, :] <= (y1 + bh)[:, None])).astype(np.float64)

    # cat layout [cores, P, chunk ci: [2, n, W]]: partition p holds rows
    # 48p..48p+47 of the per-core [BL*C*H, W] stack; chunk ci covers
    # within-partition col-groups [off, off+n), inp rows then tgt rows.
    import ml_dtypes
    f8np = ml_dtypes.float8_e4m3  # TRN fp8e4 variant (max normal 240)
    inp_r = input.reshape(N_CORES, P, RPP, W)
    tgt_r = target.reshape(N_CORES, P, RPP, W)
    n8 = sum(n for d, n in CHUNKS if d == "f8")
    n16 = sum(n for d, n in CHUNKS if d == "f16")
    cat8 = np.empty((N_CORES, P, n8 * 2 * W), dtype=f8np)
    cat16 = np.empty((N_CORES, P, n16 * 2 * W), dtype=np.float16)
    # (cat16 is skipped below and in concat when the schedule has no f16)
    pos8 = pos16 = 0
    off = 0
    for dt_, n in CHUNKS:
        if dt_ == "f8":
            cat8[:, :, pos8:pos8 + n * W] = inp_r[:, :, off:off + n
                ].reshape(N_CORES, P, n * W).astype(f8np)
            cat8[:, :, pos8 + n * W:pos8 + 2 * n * W] = tgt_r[:, :, off:off + n
                ].reshape(N_CORES, P, n * W).astype(f8np)
            pos8 += 2 * n * W
        else:
            cat16[:, :, pos16:pos16 + n * W] = inp_r[:, :, off:off + n
                ].reshape(N_CORES, P, n * W).astype(np.float16)
            cat16[:, :, pos16 + n * W:pos16 + 2 * n * W] = tgt_r[:, :, off:off + n
                ].reshape(N_CORES, P, n * W).astype(np.float16)
            pos16 += 2 * n * W
        off += n

    # ym [cores, P, RPP, 8] fp32: for col-group jj at partition p, global
    # row g = 48p + jj -> image b = g // (C*H) (== p // 32), h = g % H.
    # col 2b = ymask(h), col 2b+1 = 1.0 (only for the owning image's cols).
    pp, jj = np.meshgrid(np.arange(P), np.arange(RPP), indexing="ij")
    g = RPP * pp + jj                      # [P, RPP]
    b_idx = g // (C * H)                   # [P, RPP] in 0..BL-1
    h_idx = g % H
    ym = np.zeros((N_CORES, P, RPP, 8), dtype=np.float32)
    for k in range(N_CORES):
        ymv = ymask_full[4 * k + b_idx, h_idx]       # [P, RPP]
        np.put_along_axis(ym[k], (2 * b_idx)[..., None],
                          ymv[..., None].astype(np.float32), axis=2)
        np.put_along_axis(ym[k], (2 * b_idx + 1)[..., None],
                          np.float32(1.0), axis=2)
    # DoubleRow pair layout [cores, P, RPP//2, 2, 16], fp8 (0/1 exact),
    # cols padded to 16 for the ISA pair-step constraint
    ymp = np.zeros((N_CORES, P, RPP // 2, 2, 16), dtype=f8np)
    ymp[..., :8] = ym.reshape(N_CORES, P, RPP // 2, 2, 8).astype(f8np)
    ym = ymp

    concat = {
        "cat8": np.ascontiguousarray(cat8.reshape(N_CORES * P, n8 * 2 * W)),
        "ym": np.ascontiguousarray(ym.reshape(N_CORES * P, RPP // 2, 2, 16)),
    }
    if n16:
        concat["cat16"] = np.ascontiguousarray(
            cat16.reshape(N_CORES * P, n16 * 2 * W))

    # NB: reference mask is (B,1,H,W) — counts exclude the C factor.
    ins_cnt = float((xmask_full.sum(axis=1) * ymask_full.sum(axis=1)).sum())
    tot_cnt = float(B * H * W)
    return concat, ins_cnt, tot_cnt, xmask_full


def _run(ex, concat):
    import jax
    concat_in = [concat[name] for name in ex["in_names"]]
    zeros = [np.zeros((N_CORES * av.shape[0], *av.shape[1:]), av.dtype)
             for av in ex["out_avals"]]
    out_arrs = ex["sharded"](*concat_in, *zeros)
    out_arrs = jax.block_until_ready(out_arrs)
    return {name: np.asarray(out_arrs[i])
            for i, name in enumerate(ex["out_names"])}


def kernel(input, target, boxes):
    ex = _get_exec()
    concat, ins_cnt, tot_cnt, xmask_full = _prepare(input, target, boxes)
    outs = _run(ex, concat)

    # ycols global [8*cores, 512]: core k rows 8k+2b / 8k+2b+1 = image 4k+b
    yc = outs["ycols"].astype(np.float64).reshape(N_CORES, BL, 2, W)
    ymasked = yc[:, :, 0, :].reshape(B, W)
    tot = yc[:, :, 1, :].reshape(B, W)
    ins_sum = float((ymasked * xmask_full).sum())
    tot_sum = float(tot.sum())

    inside_loss = ins_sum / ins_cnt
    outside_loss = (tot_sum - ins_sum) / (tot_cnt - ins_cnt)
    loss = (0.5 * inside_loss + 0.5 * outside_loss) * ETA
    return np.asarray(loss, dtype=np.float32)
